# revision 38
# baseline (speedup 1.0000x reference)
"""Trainium2 Bass kernel: 3x3 same-padding conv, x[1,16,1024,1024] f32.

Strategy: shard H across 8 cores (128 output rows each; host supplies the
1-row halo by overlapping shards of a zero-padded input, so no collectives).

Perf notes (vs the f32r baseline, ~58us -> ~40us):
  - x/w/y in bf16 halves HBM traffic (in 5.8MB + out 4.2MB per core);
    PSUM accumulation stays f32, end-to-end rel err ~4e-3 vs 2e-2 gate.
  - bf16 matmuls get a separate 128-row InstLdweights each (serialized
    with the 512-col stream); matmuls are emitted kx-outer per 2-block
    PSUM group, and _shrink_dup_ldweights() rewrites every duplicate
    consecutive load to 1 row (the PE array preserves untouched rows —
    verified bit-exact on HW).  kx order zigzags between groups so each
    group's first load also dedupes.
  - output staged per block and DMA'd on the scalar ring; input chunks
    on the sync ring with small leading chunks so the PE starts early.

Per-core kernel (implicit GEMM with H-block output packing):
  - SBUF layout: partition p = u*16 + c  (u in 0..7 = input-row window slot,
    c = input channel).  Partition (u,c), slot k holds local input row
    6k+u of channel c in columns 1..1024 of a 1026-wide slot (cols 0/1025 are
    zero guards for the kx = x-shift taps).  The HOST pre-gathers the shard
    into exactly this [128, 22*1026] layout (guards included), so the input
    DMA is a flat per-partition-contiguous copy at full DMA efficiency.
  - Output rows are processed in blocks of j=6 rows: out row y = rk + j,
    rk = 6k (k=0..20) and rk=122 for the tail block k=21.
    Tap (ky,kx) of output row j needs local input row rk + (j+ky) = rk + u,
    so all 3x3 taps of a block read slot k only; u = j+ky spans 0..7.
  - matmul: out[(j,co), x] = sum_{(u,c)} lhsT_kx[(u,c),(j,co)] * X[(u,c), x+kx]
    with lhsT_kx[(u,c),(j,co)] = W[co,c,u-j,kx] if 0<=u-j<=2 else 0.
    K=128 (full contraction), M=96, N=512 (two halves per row-block).
    3 accumulating matmuls per PSUM tile (one per kx, free-dim shifted rhs).
    x/w/y are bf16 (halves HBM traffic; PSUM accumulation stays f32 and the
    measured end-to-end rel err is ~4e-3 vs the 2e-2 gate); bias stays f32.
  - PSUM -> SBUF eviction adds bias (per-partition scalar), alternating
    between DVE (tensor_scalar_add) and ACT (activation Identity+bias).
  - Output is staged 3 blocks per SBUF tile and DMA'd flat to a permuted
    y buffer [96, 22528] (host un-gathers); out-DMAs ride the second HWDGE
    ring (nc.scalar) so input and output streams don't share a FIFO.
"""

import sys

sys.path.insert(0, "/opt/trn_rl_repo")

import numpy as np

N_CORES = 8
C = 16            # channels in/out
H = 1024
W = 1024
HSH = H // N_CORES  # 128 output rows per core
HL = HSH + 2      # local input rows incl. halo
J = 6             # output rows per block
U = 8             # input-row window per block (J + 2)
SLOT = W + 2      # 1026, row slot width with zero guards
NBLK = 21         # full blocks at rk = 6k
TAIL_RK = HSH - J  # 122, tail block start
NSLOT = NBLK + 1  # 22 slots per partition
M = J * C         # 96 output partitions (j, cout)
NHALF = 512
# (slot start, n); first chunks are small so the first matmuls start early
CHUNKS = [(0, 1), (1, 1), (2, 4), (6, 4), (10, 4), (14, 4), (18, 4)]
XCOLS = NSLOT * SLOT          # 22572 per-partition input columns
NGRP = 7                      # out groups of 3 blocks (+ tail)
YCOLS = NGRP * 3 * W + W      # 22528 per-partition output columns

_CACHE = {}

# tuning knobs (also part of the build cache key)
OUT_RING = 'both'     # out-DMA initiator: 'scalar'|'sync' HWDGE rings,
                      # 'gpsimd' SWDGE (idle Pool engine, frees ACT seq),
                      # or 'both' (alternate gpsimd/scalar per block)
PS_PAIR = True        # PSUM/matmul groups of 2 blocks instead of 3
OPOOL_BUFS = 4
OG_BLOCK = True       # stage/DMA output per block (1024 cols) not per 3
EV_WIDE = True        # one [M,1024] 2-bank PSUM tile + eviction per block
                      # (requires PS_PAIR and OG_BLOCK)


def _build(reps=1, loop_n=None, parts=('in', 'mm', 'ev', 'out')):
    import contextlib

    import concourse.bacc as bacc
    import concourse.tile as tile
    import concourse.mybir as mybir

    f32 = mybir.dt.float32
    bf16 = mybir.dt.bfloat16

    nc = bacc.Bacc("TRN2", target_bir_lowering=False, debug=False,
                   num_devices=N_CORES)

    x_d = nc.dram_tensor("x", [128, XCOLS], bf16, kind="ExternalInput")
    w_d = [nc.dram_tensor(f"w{kx}", [128, M], bf16, kind="ExternalInput")
           for kx in range(3)]
    b_d = nc.dram_tensor("bvec", [M, 1], f32, kind="ExternalInput")
    y_d = nc.dram_tensor("y", [M, YCOLS], bf16, kind="ExternalOutput")

    with tile.TileContext(nc) as tc:
        with (
            tc.tile_pool(name="xpool", bufs=1) as xpool,
            tc.tile_pool(name="wpool", bufs=1) as wpool,
            tc.tile_pool(name="opool", bufs=OPOOL_BUFS) as opool,
            tc.tile_pool(name="pspool",
                         bufs=(4 if (EV_WIDE and PS_PAIR and OG_BLOCK)
                               else 8),
                         space="PSUM") as pspool,
        ):
            # weights/bias ride the scalar ring so the first input chunk
            # leads the sync ring (matters for single-shot startup)
            wt = []
            for kx in range(3):
                t = wpool.tile([128, M], bf16, tag=f"w{kx}")
                nc.scalar.dma_start(t[:], w_d[kx].ap())
                wt.append(t[:])
            bt = wpool.tile([M, 1], f32, tag="bias")
            nc.scalar.dma_start(bt[:], b_d.ap())

            ident = mybir.ActivationFunctionType.Identity
            xt = []
            for ci, (k0, ns) in enumerate(CHUNKS):
                xc = xpool.tile([128, ns * SLOT], bf16, tag=f"x{ci}")
                xt.append(xc)

            def chunk_of(k):
                for ci, (k0, ns) in enumerate(CHUNKS):
                    if k0 <= k < k0 + ns:
                        return ci, k0
                raise AssertionError(k)

            loop_cm = (tc.For_i(0, loop_n, 1) if loop_n is not None
                       else contextlib.nullcontext())
            with loop_cm:
              for _ in range(reps):
                if 'in' in parts:
                    # flat per-partition-contiguous chunk loads
                    for ci, (k0, ns) in enumerate(CHUNKS):
                        nc.sync.dma_start(
                            xt[ci][:],
                            x_d.ap()[:, k0 * SLOT:(k0 + ns) * SLOT])

                ogs = {}
                out_i = [0]

                def out_eng():
                    if OUT_RING == 'both':
                        # alternate SWDGE (gpsimd) and the scalar HWDGE
                        # ring so neither serializes the out stream
                        eng = (nc.gpsimd, nc.scalar)[out_i[0] % 2]
                        out_i[0] += 1
                        return eng
                    return {'scalar': nc.scalar, 'sync': nc.sync,
                            'gpsimd': nc.gpsimd}[OUT_RING]

                def ensure_og(g):
                    # og key: block index (OG_BLOCK) or 3-block group
                    if g not in ogs:
                        if OG_BLOCK:
                            gw = W
                            tag = "ob"
                        else:
                            gw = (3 if g < NGRP else 1) * W
                            tag = f"o{g % 2}_{gw}"
                        ogs[g] = opool.tile([M, gw], bf16, name="og",
                                            tag=tag)
                    return ogs[g]

                def flush_og(g, c0, cn):
                    og = ogs.pop(g)
                    if 'out' not in parts:
                        return
                    # out-DMAs ride their own HWDGE ring so the sync ring
                    # stays free for the next iteration's input chunks
                    if cn:   # main: full M partitions, cols [c0, c0+cn)
                        out_eng().dma_start(y_d.ap()[:, c0:c0 + cn], og[:])
                    else:    # tail block: only rows 126,127 (j=4,5) are new
                        out_eng().dma_start(
                            y_d.ap()[4 * C:6 * C, NGRP * 3 * W:],
                            og[4 * C:6 * C, :])

                if PS_PAIR:
                    mm_groups = [[2 * p, 2 * p + 1]
                                 for p in range((NSLOT + 1) // 2)]
                else:
                    mm_groups = ([[3 * g, 3 * g + 1, 3 * g + 2]
                                  for g in range(NGRP)] + [[NBLK]])

                ev_i = 0
                ev_wide = EV_WIDE and PS_PAIR and OG_BLOCK
                for gi, blocks in enumerate(mm_groups):
                    halves = [(k, h) for k in blocks for h in range(2)]
                    if ev_wide:
                        # one 2-bank [M, 1024] PSUM tile per block; each
                        # matmul still targets a single bank (512 cols)
                        psb = {k: pspool.tile([M, 2 * NHALF], f32,
                                              name="ps", tag="ps")
                               for k in blocks}
                        pss = [psb[k][:, h * NHALF:(h + 1) * NHALF]
                               for (k, h) in halves]
                    else:
                        pss = [pspool.tile([M, NHALF], f32, name="ps",
                                           tag="ps")[:]
                               for _ in range(len(halves))]
                    if 'mm' in parts:
                        # kx-outer order: the half-tiles of the group
                        # stream back-to-back against the SAME stationary
                        # tile, so the duplicate ldweights shrink to 1 row;
                        # zigzag the kx order so each group's first load
                        # repeats the previous group's last and shrinks too
                        kxs = (0, 1, 2) if gi % 2 == 0 else (2, 1, 0)
                        for i, kx in enumerate(kxs):
                            for t, (k, h) in enumerate(halves):
                                ci, k0 = chunk_of(k)
                                base = (k - k0) * SLOT + h * NHALF
                                rhs = xt[ci][:][:,
                                                base + kx:base + kx + NHALF]
                                nc.tensor.matmul(pss[t], wt[kx], rhs,
                                                 start=(i == 0),
                                                 stop=(i == 2))
                    if ev_wide:
                        for k in blocks:
                            og = ensure_og(k)
                            if 'ev' in parts:
                                if ev_i % 2 == 0:
                                    nc.vector.tensor_scalar_add(
                                        og[:], psb[k][:], bt[:])
                                else:
                                    nc.scalar.activation(og[:], psb[k][:],
                                                         ident, bias=bt[:])
                            ev_i += 1
                            if k < NBLK:
                                flush_og(k, k * W, W)
                            else:
                                flush_og(k, 0, 0)
                        continue
                    for t, (k, h) in enumerate(halves):
                        g = k if OG_BLOCK else k // 3
                        og = ensure_og(g)
                        bi = 0 if (OG_BLOCK or k == NBLK) else k - 3 * (k // 3)
                        dst_ev = og[:, bi * W + h * NHALF:
                                    bi * W + h * NHALF + NHALF]
                        if 'ev' in parts:
                            if ev_i % 2 == 0:
                                nc.vector.tensor_scalar_add(
                                    dst_ev, pss[t], bt[:])
                            else:
                                nc.scalar.activation(dst_ev, pss[t],
                                                     ident, bias=bt[:])
                        ev_i += 1
                        if h == 1:
                            if OG_BLOCK:
                                if k < NBLK:
                                    flush_og(k, k * W, W)
                                else:
                                    flush_og(k, 0, 0)
                            elif k == NBLK:
                                flush_og(k // 3, 0, 0)
                            elif k == 3 * (k // 3) + 2:
                                flush_og(k // 3, (k // 3) * 3 * W, 3 * W)

    _shrink_dup_ldweights(nc, mybir)
    nc.compile()
    return nc


def _shrink_dup_ldweights(nc, mybir):
    """Tile legalization pairs every bf16 matmul with a full 128-row
    InstLdweights (~128 PE cycles, serialized with the 512-cycle stream).
    Our kx-grouped emission makes runs of matmuls reload the SAME
    stationary tile the PE array already holds, so shrink each duplicate
    load to a single row: it rewrites row 0 with identical values, leaves
    rows 1..127 untouched, and keeps all semaphore bookkeeping intact."""
    n = 0
    for f in nc.m.functions:
        for bb in f.blocks:
            last = None
            for inst in bb.instructions:
                if not isinstance(inst, mybir.InstLdweights):
                    continue
                ap = inst.ins[0]
                sig = (ap.memref, ap.offset, str(ap.ap), str(ap.dtype))
                if sig == last:
                    pattern = [list(p) for p in ap.ap]
                    pattern[0][1] = 1
                    ap.ap = type(ap.ap)(pattern)
                    n += 1
                else:
                    last = sig
    return n


def _bf16():
    import ml_dtypes

    return np.dtype(ml_dtypes.bfloat16)


def _prep_weights(weight, bias):
    # lhsT_kx[(u,c),(j,co)] = W[co,c,u-j,kx] for 0<=u-j<=2
    wts = []
    for kx in range(3):
        wk = np.zeros((128, M), dtype=np.float32)
        for ky in range(3):
            wcc = np.ascontiguousarray(weight[:, :, ky, kx].T)  # [c, co]
            for j in range(J):
                u = j + ky
                wk[u * C:(u + 1) * C, j * C:(j + 1) * C] = wcc
        wts.append(wk.astype(_bf16()))
    bvec = np.tile(bias.astype(np.float32), J)[:, None].copy()
    return wts, bvec


def _make_in_maps(x, weight, bias):
    bf16 = _bf16()
    # zero-padded input in [row, channel, W] order, pre-rounded to bf16
    x_pad = np.zeros((H + 2, C, W), dtype=bf16)
    x_pad[1:H + 1] = x[0].transpose(1, 0, 2).astype(bf16)
    wts, bvec = _prep_weights(weight, bias)

    in_maps = []
    for s in range(N_CORES):
        # pre-gathered shard: partition p = u*16+c, slot k, cols 1..1024
        # hold local input row 6k+u (k<21) / 122+u (k=21) of channel c.
        xs = np.zeros((U, C, NSLOT, SLOT), dtype=bf16)
        r0 = s * HSH
        for u in range(U):
            # rows r0+6k+u for k=0..20 -> strided slice, [21, C, W]
            xs[u, :, :NBLK, 1:W + 1] = x_pad[
                r0 + u:r0 + u + 6 * NBLK:6].transpose(1, 0, 2)
            xs[u, :, NBLK, 1:W + 1] = x_pad[r0 + TAIL_RK + u]
        m = {"x": xs.reshape(128, XCOLS), "bvec": bvec}
        for kx in range(3):
            m[f"w{kx}"] = wts[kx]
        in_maps.append(m)
    return in_maps


def _gather_out(results):
    out = np.empty((C, H, W), dtype=np.float32)
    for s in range(N_CORES):
        yp = np.asarray(results[s]["y"]).astype(np.float32)  # [96, 22528]
        # main: rows 0..125 = (g, b, j) lexicographic
        main = yp[:, :NGRP * 3 * W].reshape(J, C, NGRP * 3, W)
        out[:, s * HSH:s * HSH + 126] = (
            main.transpose(1, 2, 0, 3).reshape(C, 126, W))
        # tail: rows 126, 127 from j = 4, 5
        tail = yp[:, NGRP * 3 * W:].reshape(J, C, W)[4:6]
        out[:, s * HSH + 126:s * HSH + 128] = tail.transpose(1, 0, 2)
    return out


def get_nc(reps=1, loop_n=None, parts=('in', 'mm', 'ev', 'out')):
    key = (f"nc{reps}_{loop_n}_{parts}_{OUT_RING}_{PS_PAIR}_{OPOOL_BUFS}"
           f"_{OG_BLOCK}_{EV_WIDE}")
    if key not in _CACHE:
        _CACHE[key] = _build(reps, loop_n, parts)
    return _CACHE[key]


def kernel(x, weight, bias):
    x = np.asarray(x, dtype=np.float32)
    weight = np.asarray(weight, dtype=np.float32)
    bias = np.asarray(bias, dtype=np.float32)

    nc = get_nc()

    from concourse.bass_utils import run_bass_kernel_spmd

    in_maps = _make_in_maps(x, weight, bias)
    res = run_bass_kernel_spmd(nc, in_maps, list(range(N_CORES)))
    return _gather_out(res.results)



# revision 39
# speedup vs baseline: 1.0172x; 1.0172x over previous
"""Trainium2 Bass kernel: 3x3 same-padding conv, x[1,16,1024,1024] f32.

Strategy: shard H across 8 cores (128 output rows each; host supplies the
1-row halo by overlapping shards of a zero-padded input, so no collectives).

Perf notes (vs the f32r baseline, ~58us -> ~40us):
  - x/w/y in bf16 halves HBM traffic (in 5.8MB + out 4.2MB per core);
    PSUM accumulation stays f32, end-to-end rel err ~4e-3 vs 2e-2 gate.
  - bf16 matmuls get a separate 128-row InstLdweights each (serialized
    with the 512-col stream); matmuls are emitted kx-outer per 2-block
    PSUM group, and _shrink_dup_ldweights() rewrites every duplicate
    consecutive load to 1 row (the PE array preserves untouched rows —
    verified bit-exact on HW).  kx order zigzags between groups so each
    group's first load also dedupes.
  - output staged per block and DMA'd on the scalar ring; input chunks
    on the sync ring with small leading chunks so the PE starts early.

Per-core kernel (implicit GEMM with H-block output packing):
  - SBUF layout: partition p = u*16 + c  (u in 0..7 = input-row window slot,
    c = input channel).  Partition (u,c), slot k holds local input row
    6k+u of channel c in columns 1..1024 of a 1026-wide slot (cols 0/1025 are
    zero guards for the kx = x-shift taps).  The HOST pre-gathers the shard
    into exactly this [128, 22*1026] layout (guards included), so the input
    DMA is a flat per-partition-contiguous copy at full DMA efficiency.
  - Output rows are processed in blocks of j=6 rows: out row y = rk + j,
    rk = 6k (k=0..20) and rk=122 for the tail block k=21.
    Tap (ky,kx) of output row j needs local input row rk + (j+ky) = rk + u,
    so all 3x3 taps of a block read slot k only; u = j+ky spans 0..7.
  - matmul: out[(j,co), x] = sum_{(u,c)} lhsT_kx[(u,c),(j,co)] * X[(u,c), x+kx]
    with lhsT_kx[(u,c),(j,co)] = W[co,c,u-j,kx] if 0<=u-j<=2 else 0.
    K=128 (full contraction), M=96, N=512 (two halves per row-block).
    3 accumulating matmuls per PSUM tile (one per kx, free-dim shifted rhs).
    x/w/y are bf16 (halves HBM traffic; PSUM accumulation stays f32 and the
    measured end-to-end rel err is ~4e-3 vs the 2e-2 gate); bias stays f32.
  - PSUM -> SBUF eviction adds bias (per-partition scalar), alternating
    between DVE (tensor_scalar_add) and ACT (activation Identity+bias).
  - Output is staged 3 blocks per SBUF tile and DMA'd flat to a permuted
    y buffer [96, 22528] (host un-gathers); out-DMAs ride the second HWDGE
    ring (nc.scalar) so input and output streams don't share a FIFO.
"""

import sys

sys.path.insert(0, "/opt/trn_rl_repo")

import numpy as np

N_CORES = 8
C = 16            # channels in/out
H = 1024
W = 1024
HSH = H // N_CORES  # 128 output rows per core
HL = HSH + 2      # local input rows incl. halo
J = 6             # output rows per block
U = 8             # input-row window per block (J + 2)
SLOT = W + 2      # 1026, row slot width with zero guards
NBLK = 21         # full blocks at rk = 6k
TAIL_RK = HSH - J  # 122, tail block start
NSLOT = NBLK + 1  # 22 slots per partition
M = J * C         # 96 output partitions (j, cout)
NHALF = 512
# (slot start, n); first chunks are small so the first matmuls start early
CHUNKS = [(0, 1), (1, 1), (2, 4), (6, 4), (10, 4), (14, 4), (18, 4)]
XCOLS = NSLOT * SLOT          # 22572 per-partition input columns
NGRP = 7                      # out groups of 3 blocks (+ tail)
YCOLS = NGRP * 3 * W + W      # 22528 per-partition output columns

_CACHE = {}

# tuning knobs (also part of the build cache key)
OUT_RING = 'both'     # out-DMA initiator: 'scalar'|'sync' HWDGE rings,
                      # 'gpsimd' SWDGE (idle Pool engine, frees ACT seq),
                      # or 'both' (alternate gpsimd/scalar per block)
PS_PAIR = True        # PSUM/matmul groups of 2 blocks instead of 3
OPOOL_BUFS = 4
OG_BLOCK = True       # stage/DMA output per block (1024 cols) not per 3
EV_WIDE = True        # one [M,1024] 2-bank PSUM tile + eviction per block
                      # (requires PS_PAIR and OG_BLOCK)


def _build(reps=1, loop_n=None, parts=('in', 'mm', 'ev', 'out')):
    import contextlib

    import concourse.bacc as bacc
    import concourse.tile as tile
    import concourse.mybir as mybir

    f32 = mybir.dt.float32
    bf16 = mybir.dt.bfloat16

    nc = bacc.Bacc("TRN2", target_bir_lowering=False, debug=False,
                   num_devices=N_CORES)

    x_d = nc.dram_tensor("x", [128, XCOLS], bf16, kind="ExternalInput")
    w_d = [nc.dram_tensor(f"w{kx}", [128, M], bf16, kind="ExternalInput")
           for kx in range(3)]
    b_d = nc.dram_tensor("bvec", [M, 1], f32, kind="ExternalInput")
    y_d = nc.dram_tensor("y", [M, YCOLS], bf16, kind="ExternalOutput")

    with tile.TileContext(nc) as tc:
        with (
            tc.tile_pool(name="xpool", bufs=1) as xpool,
            tc.tile_pool(name="wpool", bufs=1) as wpool,
            tc.tile_pool(name="opool", bufs=OPOOL_BUFS) as opool,
            tc.tile_pool(name="pspool",
                         bufs=(4 if (EV_WIDE and PS_PAIR and OG_BLOCK)
                               else 8),
                         space="PSUM") as pspool,
        ):
            # weights/bias ride the scalar ring so the first input chunk
            # leads the sync ring (matters for single-shot startup)
            wt = []
            for kx in range(3):
                t = wpool.tile([128, M], bf16, tag=f"w{kx}")
                nc.scalar.dma_start(t[:], w_d[kx].ap())
                wt.append(t[:])
            bt = wpool.tile([M, 1], f32, tag="bias")
            nc.scalar.dma_start(bt[:], b_d.ap())

            ident = mybir.ActivationFunctionType.Identity
            xt = []
            for ci, (k0, ns) in enumerate(CHUNKS):
                xc = xpool.tile([128, ns * SLOT], bf16, tag=f"x{ci}")
                xt.append(xc)

            def chunk_of(k):
                for ci, (k0, ns) in enumerate(CHUNKS):
                    if k0 <= k < k0 + ns:
                        return ci, k0
                raise AssertionError(k)

            loop_cm = (tc.For_i(0, loop_n, 1) if loop_n is not None
                       else contextlib.nullcontext())
            with loop_cm:
              for _ in range(reps):
                if 'in' in parts:
                    # flat per-partition-contiguous chunk loads
                    for ci, (k0, ns) in enumerate(CHUNKS):
                        nc.sync.dma_start(
                            xt[ci][:],
                            x_d.ap()[:, k0 * SLOT:(k0 + ns) * SLOT])

                ogs = {}
                out_i = [0]

                def out_eng():
                    if OUT_RING == 'both':
                        # alternate SWDGE (gpsimd) and the scalar HWDGE
                        # ring so neither serializes the out stream
                        eng = (nc.gpsimd, nc.scalar)[out_i[0] % 2]
                        out_i[0] += 1
                        return eng
                    return {'scalar': nc.scalar, 'sync': nc.sync,
                            'gpsimd': nc.gpsimd}[OUT_RING]

                def ensure_og(g):
                    # og key: block index (OG_BLOCK) or 3-block group
                    if g not in ogs:
                        if OG_BLOCK:
                            gw = W
                            tag = "ob"
                        else:
                            gw = (3 if g < NGRP else 1) * W
                            tag = f"o{g % 2}_{gw}"
                        ogs[g] = opool.tile([M, gw], bf16, name="og",
                                            tag=tag)
                    return ogs[g]

                def flush_og(g, c0, cn):
                    og = ogs.pop(g)
                    if 'out' not in parts:
                        return
                    # out-DMAs ride their own HWDGE ring so the sync ring
                    # stays free for the next iteration's input chunks
                    if cn:   # main: full M partitions, cols [c0, c0+cn)
                        out_eng().dma_start(y_d.ap()[:, c0:c0 + cn], og[:])
                    else:    # tail block: only rows 126,127 (j=4,5) are new
                        out_eng().dma_start(
                            y_d.ap()[4 * C:6 * C, NGRP * 3 * W:],
                            og[4 * C:6 * C, :])

                if PS_PAIR:
                    mm_groups = [[2 * p, 2 * p + 1]
                                 for p in range((NSLOT + 1) // 2)]
                else:
                    mm_groups = ([[3 * g, 3 * g + 1, 3 * g + 2]
                                  for g in range(NGRP)] + [[NBLK]])

                ev_i = 0
                ev_wide = EV_WIDE and PS_PAIR and OG_BLOCK
                for gi, blocks in enumerate(mm_groups):
                    halves = [(k, h) for k in blocks for h in range(2)]
                    if ev_wide:
                        # one 2-bank [M, 1024] PSUM tile per block; each
                        # matmul still targets a single bank (512 cols)
                        psb = {k: pspool.tile([M, 2 * NHALF], f32,
                                              name="ps", tag="ps")
                               for k in blocks}
                        pss = [psb[k][:, h * NHALF:(h + 1) * NHALF]
                               for (k, h) in halves]
                    else:
                        pss = [pspool.tile([M, NHALF], f32, name="ps",
                                           tag="ps")[:]
                               for _ in range(len(halves))]
                    if 'mm' in parts:
                        # kx-outer order: the half-tiles of the group
                        # stream back-to-back against the SAME stationary
                        # tile, so the duplicate ldweights shrink to 1 row;
                        # zigzag the kx order so each group's first load
                        # repeats the previous group's last and shrinks too
                        kxs = (0, 1, 2) if gi % 2 == 0 else (2, 1, 0)
                        for i, kx in enumerate(kxs):
                            for t, (k, h) in enumerate(halves):
                                ci, k0 = chunk_of(k)
                                base = (k - k0) * SLOT + h * NHALF
                                rhs = xt[ci][:][:,
                                                base + kx:base + kx + NHALF]
                                nc.tensor.matmul(pss[t], wt[kx], rhs,
                                                 start=(i == 0),
                                                 stop=(i == 2))
                    if ev_wide:
                        for k in blocks:
                            og = ensure_og(k)
                            if 'ev' in parts:
                                if k == NBLK:
                                    # tail: only rows 126,127 (j=4,5) are
                                    # read, and this eviction ends the
                                    # iteration's serial chain
                                    src = psb[k][4 * C:6 * C, :]
                                    dst = og[4 * C:6 * C, :]
                                    bts = bt[4 * C:6 * C, :]
                                else:
                                    src, dst, bts = psb[k][:], og[:], bt[:]
                                if ev_i % 2 == 0:
                                    nc.vector.tensor_scalar_add(
                                        dst, src, bts)
                                else:
                                    nc.scalar.activation(dst, src,
                                                         ident, bias=bts)
                            ev_i += 1
                            if k < NBLK:
                                flush_og(k, k * W, W)
                            else:
                                flush_og(k, 0, 0)
                        continue
                    for t, (k, h) in enumerate(halves):
                        g = k if OG_BLOCK else k // 3
                        og = ensure_og(g)
                        bi = 0 if (OG_BLOCK or k == NBLK) else k - 3 * (k // 3)
                        dst_ev = og[:, bi * W + h * NHALF:
                                    bi * W + h * NHALF + NHALF]
                        if 'ev' in parts:
                            if ev_i % 2 == 0:
                                nc.vector.tensor_scalar_add(
                                    dst_ev, pss[t], bt[:])
                            else:
                                nc.scalar.activation(dst_ev, pss[t],
                                                     ident, bias=bt[:])
                        ev_i += 1
                        if h == 1:
                            if OG_BLOCK:
                                if k < NBLK:
                                    flush_og(k, k * W, W)
                                else:
                                    flush_og(k, 0, 0)
                            elif k == NBLK:
                                flush_og(k // 3, 0, 0)
                            elif k == 3 * (k // 3) + 2:
                                flush_og(k // 3, (k // 3) * 3 * W, 3 * W)

    _shrink_dup_ldweights(nc, mybir)
    nc.compile()
    return nc


def _shrink_dup_ldweights(nc, mybir):
    """Tile legalization pairs every bf16 matmul with a full 128-row
    InstLdweights (~128 PE cycles, serialized with the 512-cycle stream).
    Our kx-grouped emission makes runs of matmuls reload the SAME
    stationary tile the PE array already holds, so shrink each duplicate
    load to a single row: it rewrites row 0 with identical values, leaves
    rows 1..127 untouched, and keeps all semaphore bookkeeping intact."""
    n = 0
    for f in nc.m.functions:
        for bb in f.blocks:
            last = None
            for inst in bb.instructions:
                if not isinstance(inst, mybir.InstLdweights):
                    continue
                ap = inst.ins[0]
                sig = (ap.memref, ap.offset, str(ap.ap), str(ap.dtype))
                if sig == last:
                    pattern = [list(p) for p in ap.ap]
                    pattern[0][1] = 1
                    ap.ap = type(ap.ap)(pattern)
                    n += 1
                else:
                    last = sig
    return n


def _bf16():
    import ml_dtypes

    return np.dtype(ml_dtypes.bfloat16)


def _prep_weights(weight, bias):
    # lhsT_kx[(u,c),(j,co)] = W[co,c,u-j,kx] for 0<=u-j<=2
    wts = []
    for kx in range(3):
        wk = np.zeros((128, M), dtype=np.float32)
        for ky in range(3):
            wcc = np.ascontiguousarray(weight[:, :, ky, kx].T)  # [c, co]
            for j in range(J):
                u = j + ky
                wk[u * C:(u + 1) * C, j * C:(j + 1) * C] = wcc
        wts.append(wk.astype(_bf16()))
    bvec = np.tile(bias.astype(np.float32), J)[:, None].copy()
    return wts, bvec


def _make_in_maps(x, weight, bias):
    bf16 = _bf16()
    # zero-padded input in [row, channel, W] order, pre-rounded to bf16
    x_pad = np.zeros((H + 2, C, W), dtype=bf16)
    x_pad[1:H + 1] = x[0].transpose(1, 0, 2).astype(bf16)
    wts, bvec = _prep_weights(weight, bias)

    in_maps = []
    for s in range(N_CORES):
        # pre-gathered shard: partition p = u*16+c, slot k, cols 1..1024
        # hold local input row 6k+u (k<21) / 122+u (k=21) of channel c.
        xs = np.zeros((U, C, NSLOT, SLOT), dtype=bf16)
        r0 = s * HSH
        for u in range(U):
            # rows r0+6k+u for k=0..20 -> strided slice, [21, C, W]
            xs[u, :, :NBLK, 1:W + 1] = x_pad[
                r0 + u:r0 + u + 6 * NBLK:6].transpose(1, 0, 2)
            xs[u, :, NBLK, 1:W + 1] = x_pad[r0 + TAIL_RK + u]
        m = {"x": xs.reshape(128, XCOLS), "bvec": bvec}
        for kx in range(3):
            m[f"w{kx}"] = wts[kx]
        in_maps.append(m)
    return in_maps


def _gather_out(results):
    out = np.empty((C, H, W), dtype=np.float32)
    for s in range(N_CORES):
        yp = np.asarray(results[s]["y"]).astype(np.float32)  # [96, 22528]
        # main: rows 0..125 = (g, b, j) lexicographic
        main = yp[:, :NGRP * 3 * W].reshape(J, C, NGRP * 3, W)
        out[:, s * HSH:s * HSH + 126] = (
            main.transpose(1, 2, 0, 3).reshape(C, 126, W))
        # tail: rows 126, 127 from j = 4, 5
        tail = yp[:, NGRP * 3 * W:].reshape(J, C, W)[4:6]
        out[:, s * HSH + 126:s * HSH + 128] = tail.transpose(1, 0, 2)
    return out


def get_nc(reps=1, loop_n=None, parts=('in', 'mm', 'ev', 'out')):
    key = (f"nc{reps}_{loop_n}_{parts}_{OUT_RING}_{PS_PAIR}_{OPOOL_BUFS}"
           f"_{OG_BLOCK}_{EV_WIDE}")
    if key not in _CACHE:
        _CACHE[key] = _build(reps, loop_n, parts)
    return _CACHE[key]


def kernel(x, weight, bias):
    x = np.asarray(x, dtype=np.float32)
    weight = np.asarray(weight, dtype=np.float32)
    bias = np.asarray(bias, dtype=np.float32)

    nc = get_nc()

    from concourse.bass_utils import run_bass_kernel_spmd

    in_maps = _make_in_maps(x, weight, bias)
    res = run_bass_kernel_spmd(nc, in_maps, list(range(N_CORES)))
    return _gather_out(res.results)

